# revision 10
# baseline (speedup 1.0000x reference)
"""Trainium2 Bass kernel for nn_CQFusion — v2 (bf16, XBAR-transpose, folded rank-1).

Math (per batch, all-ones masks, zero bias):
    S[c,q]   = ctx@w4C |c  +  qry@w4Q |q  +  (ctx*w4mlu)@qry^T
    A        = softmax_q(S),  Bt = softmax_c(S)
    c2q      = A @ qry
    q2c      = A @ (Bt^T @ ctx)
    out      = [ctx | c2q | ctx*c2q | ctx*q2c] @ W^T

Key identities exploited:
  - S = S0 + cw[c] + qw[q] with S0 the bilinear term. exp(S) = exp(S0)*ecw*eqw.
    ecw cancels in A (row softmax), eqw cancels in Bt, so E0 = exp(S0) is the
    only exp array: eqw folds into the q-side matmul operands (per-partition
    scalars), ecw into the VT lhsT. Normalizers are weighted sums of E0,
    computed as N=1 "mini" matmuls against ecw/eqw columns.
  - E0^T comes from XBAR DMA transposes (bf16), not PE transposes or recompute.
  - A's normalizer 1/rs is applied post-projection: projection runs
    "transposed" (out[c,e] per 128-c tile) so 1/rs is a per-partition scalar,
    fused with the (W0-term + rest) merge in one scalar_tensor_tensor per tile.
  - q2c = A @ (Bt^T ctx) re-associated; Bt^T ctx built from VT = (ctx*ecw)^T E0
    via XBAR transpose, scaled by csi*eqw.

Layouts (host pre-packs one bf16 blob per batch in SBUF image form):
  Cn16  [c-par, (ct,d)]   ctx natural      CT16 [d-par, c]  ctx transposed
  QT16  [d-par, q]        qry transposed   Qn16 [q-par, (qt,d)] qry natural
Output device layout: [c-par, (ct, e)] bf16; host reassembles + adds b/c_mask.
"""

import numpy as np
import ml_dtypes

import concourse.bass as bass
import concourse.bacc as bacc
import concourse.tile as tile
from concourse import mybir
from concourse.bass_utils import run_bass_kernel_spmd

F32 = mybir.dt.float32
BF16 = mybir.dt.bfloat16
EXP = mybir.ActivationFunctionType.Exp
MUL = mybir.AluOpType.mult
ADD = mybir.AluOpType.add
ts = bass.ts
NPBF = ml_dtypes.bfloat16

B, Lc, Lq, D = 16, 2048, 512, 128
NCORES = 8
BPC = B // NCORES   # batches per core
NTC = Lc // 128     # 16 c-tiles
NTQ = Lq // 128     # 4 q-tiles
NCH = Lc // 512     # 4 c-chunks
BLOB = 2 * Lc + Lq + Lq   # bf16 elems per partition: Cn16, CT16, QT16, Qn16


WA = Lc + 2 * Lq + 2
A1W = 512 + Lq + 2


def _load(nc, pools, blobA_d, blobB_d, b):
    big = pools[1]
    blobA = big.tile([128, WA], BF16, tag="blobA")
    nc.sync.dma_start(blobA[:], blobA_d.ap()[b * 128:(b + 1) * 128, :])
    blobB = big.tile([128, Lc + Lq], BF16, tag="blobB")
    nc.sync.dma_start(blobB[:], blobB_d.ap()[b * 128:(b + 1) * 128, :])
    # blobA layout: [CT(0:512) | QMT | w4c | w4q | CT(512:2048) | QT16]
    return {"CTa": blobA[:, 0:512], "QMT": blobA[:, 512:512 + Lq],
            "w4c": blobA[:, 512 + Lq:512 + Lq + 1],
            "w4q": blobA[:, 512 + Lq + 1:512 + Lq + 2],
            "CTb": blobA[:, A1W:A1W + 1536],
            "QT16": blobA[:, A1W + 1536:A1W + 1536 + Lq],
            "Cn16": blobB[:, 0:Lc], "Qn16": blobB[:, Lc:Lc + Lq]}


def _ct(st, t, n=128):
    # CT16 tile t columns [t*128, t*128+n) across the A1/A2 split
    if (t + 1) * 128 <= 512:
        return st["CTa"][:, t * 128:t * 128 + n]
    return st["CTb"][:, t * 128 - 512:t * 128 - 512 + n]


def _front(nc, pools, consts, st, filler=None):
    sbuf, big, bp16p, psS, psVT, psUQ = pools
    wb = consts[0]
    QT16, Cn16, Qn16 = st["QT16"], st["Cn16"], st["Qn16"]
    QMT = st["QMT"]
    w4c, w4q = st["w4c"], st["w4q"]

    cwc = psS.tile([128, NTC], F32, tag="sg")
    for t in range(NTC):
        nc.tensor.matmul(cwc[:, t:t + 1], _ct(st, t), w4c[:],
                         skip_group_check=True)
    qwc = psS.tile([128, NTQ], F32, tag="sg")
    for qt in range(NTQ):
        nc.tensor.matmul(qwc[:, qt:qt + 1], QT16[:, ts(qt, 128)], w4q[:],
                         skip_group_check=True)
    ecw = sbuf.tile([128, NTC], F32, tag="ecw")
    nc.scalar.activation(ecw[:], cwc[:], EXP)
    eqw = sbuf.tile([128, NTQ], F32, tag="eqw")
    nc.scalar.activation(eqw[:], qwc[:], EXP)
    ecw16 = sbuf.tile([128, NTC], BF16, tag="ecw16")
    nc.vector.tensor_copy(ecw16[:], ecw[:])
    eqw16 = sbuf.tile([128, NTQ], BF16, tag="eqw16")
    nc.vector.tensor_copy(eqw16[:], eqw[:])

    Qse = sbuf.tile([128, Lq], BF16, tag="Qse")
    for qt in range(NTQ):
        nc.gpsimd.tensor_scalar_mul(Qse[:, ts(qt, 128)], Qn16[:, ts(qt, 128)],
                                    eqw[:, qt:qt + 1])
    Cne = sbuf.tile([128, Lc], BF16, tag="Cne")
    for t in range(NTC):
        nc.gpsimd.tensor_scalar_mul(Cne[:, ts(t, 128)], Cn16[:, ts(t, 128)],
                                    ecw[:, t:t + 1])

    E16 = big.tile([128, NTC * 512], BF16, tag="E16")
    vtp = psVT.tile([128, 512], F32, tag="vt")
    LAG = 2
    for g in range(8):
        sp = psS.tile([128, 1024], F32, tag="sg")
        for j in range(2):
            t = 2 * g + j
            nc.tensor.matmul(sp[:, ts(j, 512)], _ct(st, t), QMT[:])
        nc.scalar.activation(E16[:, g * 1024:(g + 1) * 1024], sp[:], EXP)
        if filler is not None and g % 2 == 1:
            filler(g // 2)
        if g >= LAG:
            v = g - LAG
            for j in range(2):
                t = 2 * v + j
                nc.tensor.matmul(vtp[:], Cne[:, ts(t, 128)], E16[:, ts(t, 512)],
                                 start=(t == 0), stop=False)
    for v in range(8 - LAG, 8):
        for j in range(2):
            t = 2 * v + j
            nc.tensor.matmul(vtp[:], Cne[:, ts(t, 128)], E16[:, ts(t, 512)],
                             start=False, stop=(t == NTC - 1))

    ET16 = big.tile([128, NTC * 512], BF16, tag="ET16")
    ETv = ET16[:].rearrange("p (t h c) -> p (t h) c", c=128, h=NTQ)
    for g in range(8):
        nc.sync.dma_start_transpose(
            ETv[:, g * 8:(g + 1) * 8, :],
            E16[:, g * 1024:(g + 1) * 1024])

    csc = psS.tile([128, NTQ], F32, tag="sg")
    for qs in range(NTQ):
        for t in range(NTC):
            nc.tensor.matmul(csc[:, qs:qs + 1],
                             E16[:, t * 512 + qs * 128:t * 512 + (qs + 1) * 128],
                             ecw16[:, t:t + 1],
                             start=(t == 0), stop=(t == NTC - 1),
                             skip_group_check=True)

    csi = sbuf.tile([128, NTQ], F32, tag="csi")
    nc.vector.reciprocal(csi[:], csc[:])
    csie = sbuf.tile([128, NTQ], F32, tag="csie")
    nc.vector.tensor_mul(csie[:], csi[:], eqw[:])
    VT16 = sbuf.tile([128, 512], BF16, tag="VT16")
    nc.vector.tensor_copy(VT16[:], vtp[:])
    TMPT = sbuf.tile([128, 512], BF16, tag="TMPT")
    nc.sync.dma_start_transpose(
        TMPT[:].rearrange("p (t c) -> p t c", c=128), VT16[:])
    TMPe = sbuf.tile([128, 512], BF16, tag="TMPe")
    for qt in range(NTQ):
        nc.gpsimd.tensor_scalar_mul(TMPe[:, ts(qt, 128)], TMPT[:, ts(qt, 128)],
                                    csie[:, qt:qt + 1])

    st.update(E16=E16, ET16=ET16, TMPe=TMPe, Qse=Qse, eqw16=eqw16)


def _midA_bp(nc, pools, consts, st):
    sbuf, big, bp16p, psS, psVT, psUQ = pools
    wb = consts[0]
    BP16s = []
    for ch in range(NCH):
        bpp = psVT.tile([128, 512], F32, tag="vt")
        for sub in range(4):
            t = 4 * ch + sub
            cs_ = slice(sub * 128, (sub + 1) * 128)
            nc.tensor.matmul(bpp[:, cs_], _ct(st, t), wb[:, ts(0, 128)])
        BP16 = bp16p.tile([128, 512], BF16, tag="BP16")
        nc.vector.tensor_copy(BP16[:], bpp[:])
        BP16s.append(BP16)
    st["BP16s"] = BP16s


def _et_rhs(st, ch, qt):
    return st["ET16"][:].rearrange("p (t h c) -> p t h c", c=128, h=NTQ)[
        :, 4 * ch:4 * ch + 4, qt, :]


def _midA_chunk(nc, pools, st, ch, b):
    sbuf, big, bp16p, psS, psVT, psUQ = pools
    Qse = st["Qse"]
    utp = psUQ.tile([128, 512], F32, tag="uq")
    for qt in range(NTQ):
        nc.tensor.matmul(utp[:], Qse[:, ts(qt, 128)], _et_rhs(st, ch, qt),
                         start=(qt == 0), stop=(qt == NTQ - 1))
    U16 = sbuf.tile([128, 512], BF16, tag=f"U16_{ch}")
    nc.vector.tensor_copy(U16[:], utp[:])
    P3 = sbuf.tile([128, 512], BF16, tag=f"P3_{ch}")
    nc.gpsimd.tensor_mul(P3[:], _ct(st, 4 * ch, 512), U16[:])
    st.setdefault("U16s", {})[ch] = U16
    st.setdefault("P3s", {})[ch] = P3


def _midA_tail(nc, pools, st):
    sbuf, big, bp16p, psS, psVT, psUQ = pools
    rsc = psS.tile([128, NTC], F32, tag="sg")
    for t in range(NTC):
        for qh in range(NTQ):
            nc.tensor.matmul(rsc[:, t:t + 1],
                             st["ET16"][:, (t * NTQ + qh) * 128:(t * NTQ + qh + 1) * 128],
                             st["eqw16"][:, qh:qh + 1],
                             start=(qh == 0), stop=(qh == NTQ - 1),
                             skip_group_check=True)
    rsi = sbuf.tile([128, NTC], F32, tag="rsi")
    nc.vector.reciprocal(rsi[:], rsc[:])
    st["rsi"] = rsi


def _back_alloc(nc, pools, st):
    OUT = pools[1].tile([128, NTC * 128], BF16, tag="OUT")
    st["OUT"] = OUT


def _back_chunk(nc, pools, consts, st, out_d, b, ch):
    sbuf, big, bp16p, psS, psVT, psUQ = pools
    wb = consts[0]
    TMPe, rsi = st["TMPe"], st["rsi"]
    OUT = st["OUT"]
    if True:
        U16, P3 = st["U16s"][ch], st["P3s"][ch]
        q2p = psUQ.tile([128, 512], F32, tag="uq")
        for qt in range(NTQ):
            nc.tensor.matmul(q2p[:], TMPe[:, ts(qt, 128)], _et_rhs(st, ch, qt),
                             start=(qt == 0), stop=(qt == NTQ - 1))
        P4 = sbuf.tile([128, 512], BF16, tag=f"P4_{ch}")
        nc.vector.tensor_mul(P4[:], _ct(st, 4 * ch, 512), q2p[:])

        app = psS.tile([128, 1024], F32, tag="sg")
        for sub in range(4):
            t = 4 * ch + sub
            cs_ = slice(sub * 128, (sub + 1) * 128)
            nc.tensor.matmul(app[:, cs_], U16[:, cs_], wb[:, ts(1, 128)],
                             start=True, stop=False)
            nc.tensor.matmul(app[:, cs_], P3[:, cs_], wb[:, ts(2, 128)],
                             start=False, stop=False)
            nc.tensor.matmul(app[:, cs_], P4[:, cs_], wb[:, ts(3, 128)],
                             start=False, stop=True)
        for sub in range(4):
            t = 4 * ch + sub
            cs_ = slice(sub * 128, (sub + 1) * 128)
            nc.vector.scalar_tensor_tensor(
                OUT[:, t * 128:(t + 1) * 128], app[:, cs_], rsi[:, t:t + 1],
                st["BP16s"][ch][:, cs_], MUL, ADD)
        nc.sync.dma_start(
            out_d.ap()[b * 128:(b + 1) * 128, ch * 512:(ch + 1) * 512],
            OUT[:, ts(ch, 512)])


def _emit(ctx_es, tc, nc, blobA_d, blobB_d, cst_d, out_d):
    sbuf = ctx_es.enter_context(tc.tile_pool(name="sbuf", bufs=2))
    big = ctx_es.enter_context(tc.tile_pool(name="big", bufs=2))
    cst = ctx_es.enter_context(tc.tile_pool(name="cst", bufs=1))
    bp16p = ctx_es.enter_context(tc.tile_pool(name="bp16p", bufs=8))
    psS = ctx_es.enter_context(tc.tile_pool(name="psS", bufs=2, space="PSUM"))
    psVT = ctx_es.enter_context(tc.tile_pool(name="psVT", bufs=2, space="PSUM"))
    psUQ = ctx_es.enter_context(tc.tile_pool(name="psUQ", bufs=2, space="PSUM"))

    pools = (sbuf, big, bp16p, psS, psVT, psUQ)
    big2 = pools[1]
    blobA0 = big2.tile([128, WA], BF16, tag="blobA")
    nc.sync.dma_start(blobA0[:], blobA_d.ap()[0:128, :])
    cstt = cst.tile([128, 4 * 128], BF16, tag="cstt")
    nc.sync.dma_start(cstt[:], cst_d.ap())
    wb = cstt[:, 0:512]
    blobB0 = big2.tile([128, Lc + Lq], BF16, tag="blobB")
    nc.sync.dma_start(blobB0[:], blobB_d.ap()[0:128, :])
    st0 = {"CTa": blobA0[:, 0:512], "QMT": blobA0[:, 512:512 + Lq],
           "w4c": blobA0[:, 512 + Lq:512 + Lq + 1],
           "w4q": blobA0[:, 512 + Lq + 1:512 + Lq + 2],
           "CTb": blobA0[:, A1W:A1W + 1536],
           "QT16": blobA0[:, A1W + 1536:A1W + 1536 + Lq],
           "Cn16": blobB0[:, 0:Lc], "Qn16": blobB0[:, Lc:Lc + Lq]}

    consts = (wb,)
    sts = [st0] + [_load(nc, pools, blobA_d, blobB_d, b) for b in range(1, BPC)]
    _front(nc, pools, consts, sts[0])
    _front(nc, pools, consts, sts[1])
    _midA_bp(nc, pools, consts, sts[0])
    for ch in range(NCH):
        _midA_chunk(nc, pools, sts[0], ch, 0)
    _midA_tail(nc, pools, sts[0])
    _midA_bp(nc, pools, consts, sts[1])
    for ch in range(NCH):
        _midA_chunk(nc, pools, sts[1], ch, 1)
    _midA_tail(nc, pools, sts[1])
    for b in range(BPC):
        _back_alloc(nc, pools, sts[b])
    for ch in range(NCH):
        for b in range(BPC):
            _back_chunk(nc, pools, consts, sts[b], out_d, b, ch)


def build_nc():
    from contextlib import ExitStack

    nc = bacc.Bacc("TRN2", target_bir_lowering=False, debug=False,
                   num_devices=NCORES)
    blobA_d = nc.dram_tensor("blobA", [BPC * 128, Lc + 2 * Lq + 2], BF16, kind="ExternalInput")
    blobB_d = nc.dram_tensor("blobB", [BPC * 128, Lc + Lq], BF16, kind="ExternalInput")
    cst_d = nc.dram_tensor("cstb", [128, 4 * 128], BF16, kind="ExternalInput")
    out_d = nc.dram_tensor("out", [BPC * 128, NTC * 128], BF16, kind="ExternalOutput")

    with tile.TileContext(nc) as tc:
        with ExitStack() as ctx_es:
            _emit(ctx_es, tc, nc, blobA_d, blobB_d, cst_d, out_d)
    nc.compile()
    return nc


_NC_CACHE = None


def _get_nc():
    global _NC_CACHE
    if _NC_CACHE is None:
        _NC_CACHE = build_nc()
    return _NC_CACHE


def _in_maps(context, query, w4C, w4Q, w4mlu, W):
    ctxf = np.asarray(context, np.float32)
    qryf = np.asarray(query, np.float32)
    w4mf = np.asarray(w4mlu, np.float32).reshape(1, D)
    WA = Lc + 2 * Lq + 2
    blobsA = np.empty((B, 128, WA), NPBF)
    blobsB = np.empty((B, 128, Lc + Lq), NPBF)
    for b in range(B):
        c = ctxf[b]; q = qryf[b]
        cT = c.T
        blobsA[b, :, 0:512] = cT[:, 0:512]
        blobsA[b, :, 512:512 + Lq] = (q * w4mf).T
        blobsA[b, :, 512 + Lq] = np.asarray(w4C, np.float32).reshape(D)
        blobsA[b, :, 512 + Lq + 1] = np.asarray(w4Q, np.float32).reshape(D)
        blobsA[b, :, 512 + Lq + 2:512 + Lq + 2 + 1536] = cT[:, 512:2048]
        blobsA[b, :, 512 + Lq + 2 + 1536:] = q.T
        blobsB[b, :, 0:Lc] = c.reshape(NTC, 128, D).transpose(1, 0, 2).reshape(128, Lc)
        blobsB[b, :, Lc:] = q.reshape(NTQ, 128, D).transpose(1, 0, 2).reshape(128, Lq)
    Wf = np.asarray(W, np.float32).reshape(D, 4 * D)
    cstb = np.empty((128, 4 * 128), NPBF)
    for blk in range(4):
        cstb[:, blk * 128:(blk + 1) * 128] = Wf[:, blk * 128:(blk + 1) * 128].T
    maps = []
    for core in range(NCORES):
        sA = blobsA[core * BPC:(core + 1) * BPC]
        sB = blobsB[core * BPC:(core + 1) * BPC]
        maps.append({
            "blobA": np.ascontiguousarray(sA.reshape(BPC * 128, WA)),
            "blobB": np.ascontiguousarray(sB.reshape(BPC * 128, Lc + Lq)),
            "cstb": cstb,
        })
    return maps


def kernel(context, query, bridge=None, c_mask=None, q_mask=None,
           w4C=None, w4Q=None, w4mlu=None, W=None, b=None, **_):
    nc = _get_nc()
    maps = _in_maps(context, query, w4C, w4Q, w4mlu, W)
    res = run_bass_kernel_spmd(nc, maps, core_ids=list(range(NCORES)))
    # device output [BPC*128, (ct, e)] bf16 per core -> [B, Lc, D]
    parts = []
    for i in range(NCORES):
        o = np.asarray(res.results[i]["out"]).astype(np.float32)
        o = o.reshape(BPC, 128, NTC, D).transpose(0, 2, 1, 3).reshape(BPC, Lc, D)
        parts.append(o)
    out = np.concatenate(parts, axis=0)
    if b is not None:
        out = out + np.asarray(b, np.float32).reshape(1, 1, D)
    if c_mask is not None:
        out = out * np.asarray(c_mask, np.float32)[:, :, None]
    return out.astype(np.float32)


# revision 11
# speedup vs baseline: 1.0493x; 1.0493x over previous
"""Trainium2 Bass kernel for nn_CQFusion — v2 (bf16, XBAR-transpose, folded rank-1).

Math (per batch, all-ones masks, zero bias):
    S[c,q]   = ctx@w4C |c  +  qry@w4Q |q  +  (ctx*w4mlu)@qry^T
    A        = softmax_q(S),  Bt = softmax_c(S)
    c2q      = A @ qry
    q2c      = A @ (Bt^T @ ctx)
    out      = [ctx | c2q | ctx*c2q | ctx*q2c] @ W^T

Key identities exploited:
  - S = S0 + cw[c] + qw[q] with S0 the bilinear term. exp(S) = exp(S0)*ecw*eqw.
    ecw cancels in A (row softmax), eqw cancels in Bt, so E0 = exp(S0) is the
    only exp array: eqw folds into the q-side matmul operands (per-partition
    scalars), ecw into the VT lhsT. Normalizers are weighted sums of E0,
    computed as N=1 "mini" matmuls against ecw/eqw columns.
  - E0^T comes from XBAR DMA transposes (bf16), not PE transposes or recompute.
  - A's normalizer 1/rs is applied post-projection: projection runs
    "transposed" (out[c,e] per 128-c tile) so 1/rs is a per-partition scalar,
    fused with the (W0-term + rest) merge in one scalar_tensor_tensor per tile.
  - q2c = A @ (Bt^T ctx) re-associated; Bt^T ctx built from VT = (ctx*ecw)^T E0
    via XBAR transpose, scaled by csi*eqw.

Layouts (host pre-packs one bf16 blob per batch in SBUF image form):
  Cn16  [c-par, (ct,d)]   ctx natural      CT16 [d-par, c]  ctx transposed
  QT16  [d-par, q]        qry transposed   Qn16 [q-par, (qt,d)] qry natural
Output device layout: [c-par, (ct, e)] bf16; host reassembles + adds b/c_mask.
"""

import numpy as np
import ml_dtypes

import concourse.bass as bass
import concourse.bacc as bacc
import concourse.tile as tile
from concourse import mybir
from concourse.bass_utils import run_bass_kernel_spmd

F32 = mybir.dt.float32
BF16 = mybir.dt.bfloat16
EXP = mybir.ActivationFunctionType.Exp
MUL = mybir.AluOpType.mult
ADD = mybir.AluOpType.add
ts = bass.ts
NPBF = ml_dtypes.bfloat16

B, Lc, Lq, D = 16, 2048, 512, 128
NCORES = 8
BPC = B // NCORES   # batches per core
NTC = Lc // 128     # 16 c-tiles
NTQ = Lq // 128     # 4 q-tiles
NCH = Lc // 512     # 4 c-chunks
BLOB = 2 * Lc + Lq + Lq   # bf16 elems per partition: Cn16, CT16, QT16, Qn16


WA = Lc + 2 * Lq + 2
A1W = 512 + Lq + 2


def _load(nc, pools, blobA_d, blobB_d, b):
    big = pools[1]
    blobA = big.tile([128, WA], BF16, tag="blobA")
    nc.sync.dma_start(blobA[:], blobA_d.ap()[b * 128:(b + 1) * 128, :])
    blobB = big.tile([128, Lc + Lq], BF16, tag="blobB")
    nc.sync.dma_start(blobB[:], blobB_d.ap()[b * 128:(b + 1) * 128, :])
    # blobA layout: [CT(0:512) | QMT | w4c | w4q | CT(512:2048) | QT16]
    return {"CTa": blobA[:, 0:512], "QMT": blobA[:, 512:512 + Lq],
            "w4c": blobA[:, 512 + Lq:512 + Lq + 1],
            "w4q": blobA[:, 512 + Lq + 1:512 + Lq + 2],
            "CTb": blobA[:, A1W:A1W + 1536],
            "QT16": blobA[:, A1W + 1536:A1W + 1536 + Lq],
            "Cn16": blobB[:, 0:Lc], "Qn16": blobB[:, Lc:Lc + Lq]}


def _ct(st, t, n=128):
    # CT16 tile t columns [t*128, t*128+n) across the A1/A2 split
    if (t + 1) * 128 <= 512:
        return st["CTa"][:, t * 128:t * 128 + n]
    return st["CTb"][:, t * 128 - 512:t * 128 - 512 + n]


def _front(nc, pools, consts, st, filler=None):
    sbuf, big, bp16p, psS, psVT, psUQ = pools
    wb = consts[0]
    QT16, Cn16, Qn16 = st["QT16"], st["Cn16"], st["Qn16"]
    QMT = st["QMT"]
    w4c, w4q = st["w4c"], st["w4q"]

    cwc = psS.tile([128, NTC], F32, tag="sg")
    for t in range(NTC):
        nc.tensor.matmul(cwc[:, t:t + 1], _ct(st, t), w4c[:],
                         skip_group_check=True)
    qwc = psS.tile([128, NTQ], F32, tag="sg")
    for qt in range(NTQ):
        nc.tensor.matmul(qwc[:, qt:qt + 1], QT16[:, ts(qt, 128)], w4q[:],
                         skip_group_check=True)
    ecw = sbuf.tile([128, NTC], F32, tag="ecw")
    nc.scalar.activation(ecw[:], cwc[:], EXP)
    eqw = sbuf.tile([128, NTQ], F32, tag="eqw")
    nc.scalar.activation(eqw[:], qwc[:], EXP)
    ecw16 = sbuf.tile([128, NTC], BF16, tag="ecw16")
    nc.vector.tensor_copy(ecw16[:], ecw[:])
    eqw16 = sbuf.tile([128, NTQ], BF16, tag="eqw16")
    nc.vector.tensor_copy(eqw16[:], eqw[:])

    Qse = sbuf.tile([128, Lq], BF16, tag="Qse")
    for qt in range(NTQ):
        nc.gpsimd.tensor_scalar_mul(Qse[:, ts(qt, 128)], Qn16[:, ts(qt, 128)],
                                    eqw[:, qt:qt + 1])
    Cne = sbuf.tile([128, Lc], BF16, tag="Cne")
    for t in range(NTC):
        nc.gpsimd.tensor_scalar_mul(Cne[:, ts(t, 128)], Cn16[:, ts(t, 128)],
                                    ecw[:, t:t + 1])

    E16 = big.tile([128, NTC * 512], BF16, tag="E16")
    vtp = psVT.tile([128, 512], F32, tag="vt")
    LAG = 2
    for g in range(8):
        sp = psS.tile([128, 1024], F32, tag="sg")
        for j in range(2):
            t = 2 * g + j
            nc.tensor.matmul(sp[:, ts(j, 512)], _ct(st, t), QMT[:])
        nc.scalar.activation(E16[:, g * 1024:(g + 1) * 1024], sp[:], EXP)
        if filler is not None and g % 2 == 1:
            filler(g // 2)
        if g >= LAG:
            v = g - LAG
            for j in range(2):
                t = 2 * v + j
                nc.tensor.matmul(vtp[:], Cne[:, ts(t, 128)], E16[:, ts(t, 512)],
                                 start=(t == 0), stop=False)
    for v in range(8 - LAG, 8):
        for j in range(2):
            t = 2 * v + j
            nc.tensor.matmul(vtp[:], Cne[:, ts(t, 128)], E16[:, ts(t, 512)],
                             start=False, stop=(t == NTC - 1))

    ET16 = big.tile([128, NTC * 512], BF16, tag="ET16")
    ETv = ET16[:].rearrange("p (t h c) -> p (t h) c", c=128, h=NTQ)
    for g in range(8):
        nc.sync.dma_start_transpose(
            ETv[:, g * 8:(g + 1) * 8, :],
            E16[:, g * 1024:(g + 1) * 1024])

    csc = psS.tile([128, NTQ], F32, tag="sg")
    for qs in range(NTQ):
        for t in range(NTC):
            nc.tensor.matmul(csc[:, qs:qs + 1],
                             E16[:, t * 512 + qs * 128:t * 512 + (qs + 1) * 128],
                             ecw16[:, t:t + 1],
                             start=(t == 0), stop=(t == NTC - 1),
                             skip_group_check=True)

    csi = sbuf.tile([128, NTQ], F32, tag="csi")
    nc.vector.reciprocal(csi[:], csc[:])
    csie = sbuf.tile([128, NTQ], F32, tag="csie")
    nc.vector.tensor_mul(csie[:], csi[:], eqw[:])
    VT16 = sbuf.tile([128, 512], BF16, tag="VT16")
    nc.vector.tensor_copy(VT16[:], vtp[:])
    TMPT = sbuf.tile([128, 512], BF16, tag="TMPT")
    nc.sync.dma_start_transpose(
        TMPT[:].rearrange("p (t c) -> p t c", c=128), VT16[:])
    TMPe = sbuf.tile([128, 512], BF16, tag="TMPe")
    for qt in range(NTQ):
        nc.gpsimd.tensor_scalar_mul(TMPe[:, ts(qt, 128)], TMPT[:, ts(qt, 128)],
                                    csie[:, qt:qt + 1])

    BP16s = []
    for ch in range(NCH):
        bpp = psVT.tile([128, 512], F32, tag="vt")
        for sub in range(4):
            t = 4 * ch + sub
            cs_ = slice(sub * 128, (sub + 1) * 128)
            nc.tensor.matmul(bpp[:, cs_], _ct(st, t), wb[:, ts(0, 128)])
        BP16 = bp16p.tile([128, 512], BF16, tag="BP16")
        nc.vector.tensor_copy(BP16[:], bpp[:])
        BP16s.append(BP16)

    st.update(E16=E16, ET16=ET16, TMPe=TMPe, Qse=Qse, eqw16=eqw16,
              BP16s=BP16s)


def _et_rhs(st, ch, qt):
    return st["ET16"][:].rearrange("p (t h c) -> p t h c", c=128, h=NTQ)[
        :, 4 * ch:4 * ch + 4, qt, :]


def _midA_chunk(nc, pools, st, ch, b):
    sbuf, big, bp16p, psS, psVT, psUQ = pools
    Qse = st["Qse"]
    utp = psUQ.tile([128, 512], F32, tag="uq")
    for qt in range(NTQ):
        nc.tensor.matmul(utp[:], Qse[:, ts(qt, 128)], _et_rhs(st, ch, qt),
                         start=(qt == 0), stop=(qt == NTQ - 1))
    U16 = sbuf.tile([128, 512], BF16, tag=f"U16_{ch}")
    nc.vector.tensor_copy(U16[:], utp[:])
    P3 = sbuf.tile([128, 512], BF16, tag=f"P3_{ch}")
    nc.gpsimd.tensor_mul(P3[:], _ct(st, 4 * ch, 512), U16[:])
    st.setdefault("U16s", {})[ch] = U16
    st.setdefault("P3s", {})[ch] = P3


def _midA_tail(nc, pools, st):
    sbuf, big, bp16p, psS, psVT, psUQ = pools
    rsc = psS.tile([128, NTC], F32, tag="sg")
    for t in range(NTC):
        for qh in range(NTQ):
            nc.tensor.matmul(rsc[:, t:t + 1],
                             st["ET16"][:, (t * NTQ + qh) * 128:(t * NTQ + qh + 1) * 128],
                             st["eqw16"][:, qh:qh + 1],
                             start=(qh == 0), stop=(qh == NTQ - 1),
                             skip_group_check=True)
    rsi = sbuf.tile([128, NTC], F32, tag="rsi")
    nc.vector.reciprocal(rsi[:], rsc[:])
    st["rsi"] = rsi


def _back_alloc(nc, pools, st):
    OUT = pools[1].tile([128, NTC * 128], BF16, tag="OUT")
    st["OUT"] = OUT


def _back_chunk(nc, pools, consts, st, out_d, b, ch):
    sbuf, big, bp16p, psS, psVT, psUQ = pools
    wb = consts[0]
    TMPe, rsi = st["TMPe"], st["rsi"]
    OUT = st["OUT"]
    if True:
        U16, P3 = st["U16s"][ch], st["P3s"][ch]
        q2p = psUQ.tile([128, 512], F32, tag="uq")
        for qt in range(NTQ):
            nc.tensor.matmul(q2p[:], TMPe[:, ts(qt, 128)], _et_rhs(st, ch, qt),
                             start=(qt == 0), stop=(qt == NTQ - 1))
        P4 = sbuf.tile([128, 512], BF16, tag=f"P4_{ch}")
        nc.vector.tensor_mul(P4[:], _ct(st, 4 * ch, 512), q2p[:])

        app = psS.tile([128, 1024], F32, tag="sg")
        for sub in range(4):
            t = 4 * ch + sub
            cs_ = slice(sub * 128, (sub + 1) * 128)
            nc.tensor.matmul(app[:, cs_], U16[:, cs_], wb[:, ts(1, 128)],
                             start=True, stop=False)
            nc.tensor.matmul(app[:, cs_], P3[:, cs_], wb[:, ts(2, 128)],
                             start=False, stop=False)
            nc.tensor.matmul(app[:, cs_], P4[:, cs_], wb[:, ts(3, 128)],
                             start=False, stop=True)
        for sub in range(4):
            t = 4 * ch + sub
            cs_ = slice(sub * 128, (sub + 1) * 128)
            nc.vector.scalar_tensor_tensor(
                OUT[:, t * 128:(t + 1) * 128], app[:, cs_], rsi[:, t:t + 1],
                st["BP16s"][ch][:, cs_], MUL, ADD)
        nc.sync.dma_start(
            out_d.ap()[b * 128:(b + 1) * 128, ch * 512:(ch + 1) * 512],
            OUT[:, ts(ch, 512)])


def _emit(ctx_es, tc, nc, blobA_d, blobB_d, cst_d, out_d):
    sbuf = ctx_es.enter_context(tc.tile_pool(name="sbuf", bufs=2))
    big = ctx_es.enter_context(tc.tile_pool(name="big", bufs=2))
    cst = ctx_es.enter_context(tc.tile_pool(name="cst", bufs=1))
    bp16p = ctx_es.enter_context(tc.tile_pool(name="bp16p", bufs=8))
    psS = ctx_es.enter_context(tc.tile_pool(name="psS", bufs=2, space="PSUM"))
    psVT = ctx_es.enter_context(tc.tile_pool(name="psVT", bufs=2, space="PSUM"))
    psUQ = ctx_es.enter_context(tc.tile_pool(name="psUQ", bufs=2, space="PSUM"))

    pools = (sbuf, big, bp16p, psS, psVT, psUQ)
    big2 = pools[1]
    blobA0 = big2.tile([128, WA], BF16, tag="blobA")
    nc.sync.dma_start(blobA0[:], blobA_d.ap()[0:128, :])
    cstt = cst.tile([128, 4 * 128], BF16, tag="cstt")
    nc.sync.dma_start(cstt[:], cst_d.ap())
    wb = cstt[:, 0:512]
    blobB0 = big2.tile([128, Lc + Lq], BF16, tag="blobB")
    nc.sync.dma_start(blobB0[:], blobB_d.ap()[0:128, :])
    st0 = {"CTa": blobA0[:, 0:512], "QMT": blobA0[:, 512:512 + Lq],
           "w4c": blobA0[:, 512 + Lq:512 + Lq + 1],
           "w4q": blobA0[:, 512 + Lq + 1:512 + Lq + 2],
           "CTb": blobA0[:, A1W:A1W + 1536],
           "QT16": blobA0[:, A1W + 1536:A1W + 1536 + Lq],
           "Cn16": blobB0[:, 0:Lc], "Qn16": blobB0[:, Lc:Lc + Lq]}

    consts = (wb,)
    sts = [st0] + [_load(nc, pools, blobA_d, blobB_d, b) for b in range(1, BPC)]
    _front(nc, pools, consts, sts[0])
    _front(nc, pools, consts, sts[1])
    for ch in range(NCH):
        _midA_chunk(nc, pools, sts[0], ch, 0)
    _midA_tail(nc, pools, sts[0])
    for ch in range(NCH):
        _midA_chunk(nc, pools, sts[1], ch, 1)
    _midA_tail(nc, pools, sts[1])
    for b in range(BPC):
        _back_alloc(nc, pools, sts[b])
    for ch in range(NCH):
        for b in range(BPC):
            _back_chunk(nc, pools, consts, sts[b], out_d, b, ch)


def build_nc():
    from contextlib import ExitStack

    nc = bacc.Bacc("TRN2", target_bir_lowering=False, debug=False,
                   num_devices=NCORES)
    blobA_d = nc.dram_tensor("blobA", [BPC * 128, Lc + 2 * Lq + 2], BF16, kind="ExternalInput")
    blobB_d = nc.dram_tensor("blobB", [BPC * 128, Lc + Lq], BF16, kind="ExternalInput")
    cst_d = nc.dram_tensor("cstb", [128, 4 * 128], BF16, kind="ExternalInput")
    out_d = nc.dram_tensor("out", [BPC * 128, NTC * 128], BF16, kind="ExternalOutput")

    with tile.TileContext(nc) as tc:
        with ExitStack() as ctx_es:
            _emit(ctx_es, tc, nc, blobA_d, blobB_d, cst_d, out_d)
    nc.compile()
    return nc


_NC_CACHE = None


def _get_nc():
    global _NC_CACHE
    if _NC_CACHE is None:
        _NC_CACHE = build_nc()
    return _NC_CACHE


def _in_maps(context, query, w4C, w4Q, w4mlu, W):
    ctxf = np.asarray(context, np.float32)
    qryf = np.asarray(query, np.float32)
    w4mf = np.asarray(w4mlu, np.float32).reshape(1, D)
    WA = Lc + 2 * Lq + 2
    blobsA = np.empty((B, 128, WA), NPBF)
    blobsB = np.empty((B, 128, Lc + Lq), NPBF)
    for b in range(B):
        c = ctxf[b]; q = qryf[b]
        cT = c.T
        blobsA[b, :, 0:512] = cT[:, 0:512]
        blobsA[b, :, 512:512 + Lq] = (q * w4mf).T
        blobsA[b, :, 512 + Lq] = np.asarray(w4C, np.float32).reshape(D)
        blobsA[b, :, 512 + Lq + 1] = np.asarray(w4Q, np.float32).reshape(D)
        blobsA[b, :, 512 + Lq + 2:512 + Lq + 2 + 1536] = cT[:, 512:2048]
        blobsA[b, :, 512 + Lq + 2 + 1536:] = q.T
        blobsB[b, :, 0:Lc] = c.reshape(NTC, 128, D).transpose(1, 0, 2).reshape(128, Lc)
        blobsB[b, :, Lc:] = q.reshape(NTQ, 128, D).transpose(1, 0, 2).reshape(128, Lq)
    Wf = np.asarray(W, np.float32).reshape(D, 4 * D)
    cstb = np.empty((128, 4 * 128), NPBF)
    for blk in range(4):
        cstb[:, blk * 128:(blk + 1) * 128] = Wf[:, blk * 128:(blk + 1) * 128].T
    maps = []
    for core in range(NCORES):
        sA = blobsA[core * BPC:(core + 1) * BPC]
        sB = blobsB[core * BPC:(core + 1) * BPC]
        maps.append({
            "blobA": np.ascontiguousarray(sA.reshape(BPC * 128, WA)),
            "blobB": np.ascontiguousarray(sB.reshape(BPC * 128, Lc + Lq)),
            "cstb": cstb,
        })
    return maps


def kernel(context, query, bridge=None, c_mask=None, q_mask=None,
           w4C=None, w4Q=None, w4mlu=None, W=None, b=None, **_):
    nc = _get_nc()
    maps = _in_maps(context, query, w4C, w4Q, w4mlu, W)
    res = run_bass_kernel_spmd(nc, maps, core_ids=list(range(NCORES)))
    # device output [BPC*128, (ct, e)] bf16 per core -> [B, Lc, D]
    parts = []
    for i in range(NCORES):
        o = np.asarray(res.results[i]["out"]).astype(np.float32)
        o = o.reshape(BPC, 128, NTC, D).transpose(0, 2, 1, 3).reshape(BPC, Lc, D)
        parts.append(o)
    out = np.concatenate(parts, axis=0)
    if b is not None:
        out = out + np.asarray(b, np.float32).reshape(1, 1, D)
    if c_mask is not None:
        out = out * np.asarray(c_mask, np.float32)[:, :, None]
    return out.astype(np.float32)
